# revision 48
# baseline (speedup 1.0000x reference)
"""Trainium2 Bass kernel for BestRQ vector-quantization codebook lookup.

Pipeline (per NeuronCore, data-parallel over batch B=8 across 8 cores):
  x (2048,512) --LayerNorm--> xn --PE transpose--> xnT (d-major)
  t*2^13 = (projW*2^13)^T @ xn^T  (fp32 matmul; 2^13 host-prescale)
  t split: th_r = fp32r(t*2^13) (12-bit mantissa), tl13 = t*2^13 - th_r
  codebook (host-prepped): cbr = fp32r(cb); fp8 pair tensor
    cbc[..,0,:] = e4m3((cb - cbr) * 2^13), cbc[..,1,:] = e4m3(cbr)
  score*2^13 = th_r @ cbr  (fp32r)  +  DR-correction  (fp8 DoubleRow),
  BOTH accumulated into ONE psum bank: the 2^13 prescale of the main
  product matches the native (t.cl + tl.ch)*2^13 scale of the DR pair, so
  no separate correction psum / scalar rescale / DVE add is needed.
  argmax is scale-invariant, so the bias csq13 = 0.5*||c||^2*2^13 is
  subtracted by a single DVE tensor_sub reading PSUM directly, then
  per-chunk argmax via DVE max/max_index; global combine over 16 chunks.

Phase B orders matmuls h-outer/token-inner (8 token tiles x one fixed
codebook rhs slice) across all 8 PSUM banks. The codebook is stored
chunk-major ([128, chunk, htile, cols]) so each 512-column chunk is one
fully-contiguous DMA; the fp32r and fp8 streams ride different hwdge
queues (SP / Activation). NOTE: splitting ONE tile's DMA across the two
queues races (partial-region cross-queue writes) — whole tiles only.

Measured (8-core SPMD, in-NEFF repeat marginal, difference-of-medians
estimator: reproducible to <1 us): body ~1.274 ms/core, at the PE
streaming roofline for the sustained ~2.0 GHz (P0) clock: 4096 N=512 matmuls x ~256 ns + fp32
projection. DMA (~221 us) and all DVE/scalar work are hidden behind the
PE. Phase A pipelines its 64 transposes through 4 PSUM banks (bufs=1
measurably serialized) and runs the t splits on DVE; 3 DVE PSUM-reads
per strip instead of one scalar drain measured ~25 us WORSE.

Numerics: 2 label flips out of 16384 (rel err 1.11e-2), inside the 2e-2
gate. The projection must stay fp32: fp32r+residual-correction variants
were tried (input residuals via fp32r Pe.X + fp8 P.Xe at 2^15, host-
emulated to 4e-6 rms) but HW lands at 4 flips / rel err 1.978e-2 — the
PE's fp32r product-path rounding adds noise input-residual corrections
cannot cancel, leaving <2% gate margin. Not worth 33 us of the 1.29 ms.
"""

import numpy as np
import ml_dtypes

import concourse.bacc as bacc
import concourse.bass as bass
import concourse.mybir as mybir
import concourse.tile as tile
from concourse.bass_utils import run_bass_kernel_spmd
from concourse.masks import make_identity

B, L, D, H, C = 8, 2048, 512, 1024, 8192
LN_EPS = 1e-5
N_CORES = 8

TT = L // 128      # 16 token tiles
CCH = C // 512     # 16 codebook chunks
HT = H // 128      # 8 h tiles
DT = D // 128      # 4 d tiles
TOKC = L // 512    # 4 token chunks (projection)
SC = 8192.0        # 2^13 residual scale

F32 = mybir.dt.float32
F32R = mybir.dt.float32r
F8E4 = mybir.dt.float8e4
I32 = mybir.dt.int32
U32 = mybir.dt.uint32
DRM = mybir.MatmulPerfMode.DoubleRow


def build_nc(corr_pass=True, repeat=1, argmax=True, psb_bufs=5, strip_bufs=8,
             drain="scalar", upto="full", main_pass=True):
    nc = bacc.Bacc("TRN2", target_bir_lowering=False, debug=False)

    d_x = nc.dram_tensor("x", (L, D), F32, kind="ExternalInput")
    d_pwt = nc.dram_tensor("pwt", (D, H), F32, kind="ExternalInput")
    d_lnw = nc.dram_tensor("lnw", (D,), F32, kind="ExternalInput")
    d_lnb = nc.dram_tensor("lnb", (D,), F32, kind="ExternalInput")
    # codebook splits, chunk-major + h-tiled so each chunk is ONE fully
    # contiguous 16KB(8KB)-per-partition DMA read:
    # cbr[p, cc, ht, c] = fp32r(cb)[ht*128+p, cc*512+c]
    d_cbr = nc.dram_tensor("cbr", (128, CCH, HT, 512), F32R,
                           kind="ExternalInput")
    d_cbc = nc.dram_tensor("cbc", (128, CCH, HT, 2, 512), F8E4,
                           kind="ExternalInput")
    d_csq = nc.dram_tensor("csq", (C,), F32, kind="ExternalInput")
    d_lab = nc.dram_tensor("labels", (128, TT), I32, kind="ExternalOutput")

    with tile.TileContext(nc) as tc:
        with tc.tile_pool(name="consts", bufs=1) as consts, \
             tc.tile_pool(name="persist", bufs=1) as persist:

            # ---------- constants ----------
            ident = consts.tile([128, 128], F32)
            make_identity(nc, ident)
            eps_t = consts.tile([128, 1], F32)
            nc.vector.memset(eps_t, LN_EPS)
            lnw_bc = consts.tile([128, D], F32)
            nc.sync.dma_start(
                out=lnw_bc,
                in_=bass.AP(tensor=d_lnw, offset=0, ap=[[0, 128], [1, D]]))
            lnb_bc = consts.tile([128, D], F32)
            nc.sync.dma_start(
                out=lnb_bc,
                in_=bass.AP(tensor=d_lnb, offset=0, ap=[[0, 128], [1, D]]))
            chunk_off = consts.tile([128, CCH], F32)
            for j in range(CCH):
                nc.vector.memset(chunk_off[:, j:j + 1], 512.0 * j)

            # persistent t splits, tiled per (h, tok-chunk) for fine deps
            thr = [[persist.tile([128, 512], F32R, name=f"thr{h}_{tk}",
                                 tag=f"thr{h}_{tk}") for tk in range(TOKC)]
                   for h in range(HT)]
            t8 = [[persist.tile([128, 2, 512], F8E4, name=f"t8_{h}_{tk}",
                                tag=f"t8_{h}_{tk}") for tk in range(TOKC)]
                  for h in range(HT)]
            cval = [persist.tile([128, CCH], F32, name=f"cval{t}",
                                 tag=f"cval{t}") for t in range(TT)]
            cidx = [persist.tile([128, CCH], U32, name=f"cidx{t}",
                                 tag=f"cidx{t}") for t in range(TT)]

            for rep in range(repeat):
                _body(nc, tc, rep, corr_pass, ident, eps_t, lnw_bc, lnb_bc,
                      chunk_off, thr, t8, cval, cidx,
                      d_x, d_pwt, d_cbr, d_cbc, d_csq, d_lab,
                      argmax, psb_bufs, strip_bufs, drain, upto, main_pass)

    nc.compile()
    return nc


def _body(nc, tc, rep, corr_pass, ident, eps_t, lnw_bc, lnb_bc, chunk_off,
          thr, t8, cval, cidx, d_x, d_pwt, d_cbr, d_cbc, d_csq, d_lab,
          argmax=True, psb_bufs=3, strip_bufs=4, drain="scalar", upto="full",
          main_pass=True):
    # ---------- phase A: LN + transposes + projection + split ----------
    with tc.tile_pool(name=f"phA{rep}", bufs=1) as phA, \
         tc.tile_pool(name=f"ldtmp{rep}", bufs=3) as ldtmp, \
         tc.tile_pool(name=f"psA{rep}", bufs=3, space="PSUM") as psA, \
         tc.tile_pool(name=f"psTr{rep}", bufs=4, space="PSUM") as psTr:

        # proj weight, host-pre-transposed to (d, h)
        pwT = [phA.tile([128, H], F32, name=f"pwT{d}", tag=f"pwT{d}")
               for d in range(DT)]
        for d in range(DT):
            nc.sync.dma_start(out=pwT[d], in_=d_pwt[d * 128:(d + 1) * 128, :])

        # LayerNorm + transpose to xnT (d, tok)
        xnT = [phA.tile([128, L], F32, name=f"xnT{d}", tag=f"xnT{d}")
               for d in range(DT)]
        for t in range(TT):
            x_t = ldtmp.tile([128, D], F32, tag="x_t")
            nc.sync.dma_start(out=x_t, in_=d_x[t * 128:(t + 1) * 128, :])
            stats = ldtmp.tile([128, 6], F32, tag="stats")
            nc.vector.bn_stats(out=stats, in_=x_t)
            mv = ldtmp.tile([128, 2], F32, tag="mv")
            nc.vector.bn_aggr(out=mv, in_=stats)
            rstd = ldtmp.tile([128, 1], F32, tag="rstd")
            nc.scalar.activation(out=rstd, in_=mv[:, 1:2],
                                 func=mybir.ActivationFunctionType.Sqrt,
                                 bias=eps_t, scale=1.0)
            nc.vector.reciprocal(out=rstd, in_=rstd)
            xn = ldtmp.tile([128, D], F32, tag="xn")
            nc.vector.tensor_scalar(
                out=xn, in0=x_t, scalar1=mv[:, 0:1], scalar2=rstd,
                op0=mybir.AluOpType.subtract, op1=mybir.AluOpType.mult)
            nc.vector.tensor_mul(out=xn, in0=xn, in1=lnw_bc)
            nc.vector.tensor_add(out=xn, in0=xn, in1=lnb_bc)
            for d in range(DT):
                ps_tr = psTr.tile([128, 128], F32, tag="ps_tr")
                nc.tensor.transpose(ps_tr, xn[:, d * 128:(d + 1) * 128],
                                    ident)
                nc.scalar.copy(out=xnT[d][:, t * 128:(t + 1) * 128],
                               in_=ps_tr)

        # projection t^T[h_tile, tok] = sum_d pwT[d,h].T @ xnT[d, tok]
        # fp32 (exact); split each PSUM strip into fp32r + scaled-fp8.
        for tk in range(TOKC):
            for h in range(HT):
                ps_t = psA.tile([128, 512], F32, tag="ps_t")
                for d in range(DT):
                    nc.tensor.matmul(
                        ps_t,
                        lhsT=pwT[d][:, h * 128:(h + 1) * 128],
                        rhs=xnT[d][:, tk * 512:(tk + 1) * 512],
                        start=(d == 0), stop=(d == DT - 1))
                # pwt is host-scaled by 2^13, so t_f = t * 2^13. A single
                # scalar read drains the PSUM bank; the split runs on DVE
                # from the SBUF copy (3 DVE PSUM-reads instead measurably
                # hold the bank too long and cost ~25 us).
                t_f = ldtmp.tile([128, 512], F32, tag="t_f")
                nc.scalar.copy(out=t_f, in_=ps_t)
                # th_r = fp32r(t * 2^13); the split only needs
                # self-consistency (t8[0] is computed against stored thr)
                nc.vector.tensor_copy(thr[h][tk], t_f)
                # t8[:,1,:] = e4m3(t) (pair slot 1 multiplies ch8)
                nc.vector.tensor_scalar(
                    out=t8[h][tk][:, 1, :], in0=t_f, scalar1=1.0 / SC,
                    scalar2=None, op0=mybir.AluOpType.mult)
                # tl * 2^13 = t_f - th_r, exact in fp32
                tlt = ldtmp.tile([128, 512], F32, tag="tlt")
                nc.vector.tensor_sub(out=tlt, in0=t_f,
                                     in1=thr[h][tk].bitcast(F32))
                # t8[:,0,:] = e4m3(tl * 2^13) (pair slot 0, times cl8s)
                nc.scalar.copy(out=t8[h][tk][:, 0, :], in_=tlt)

    if upto == "A":
        return
    # ---------- phase B: cross matmul + per-chunk argmax ----------
    # Main fp32r matmuls produce t.c * 2^13 (pwt host-scaled); the fp8 DR
    # correction matmuls natively produce (t.cl + tl.ch) * 2^13, so both
    # accumulate into ONE psum bank. argmax is scale-invariant, so the bias
    # is host-prescaled (csq13 = 0.5*||c||^2 * 2^13) and never rescaled:
    # the only drain is a single DVE sub reading PSUM.
    with tc.tile_pool(name=f"strips{rep}", bufs=strip_bufs) as strips, \
         tc.tile_pool(name=f"cbf{rep}", bufs=2) as cbf_pool, \
         tc.tile_pool(name=f"psB{rep}", bufs=psb_bufs, space="PSUM") as psB:

        for cc in range(CCH):
            # split each chunk's streams across BOTH hwdge queues (SP and
            # Activation) — a single queue's bandwidth gates the kernel
            cbt = cbf_pool.tile([128, HT, 512], F32R, name="cbt", tag="cbt")
            nc.sync.dma_start(out=cbt, in_=d_cbr[:, cc, :, :])
            cct = cbf_pool.tile([128, HT, 2, 512], F8E4, name="cct",
                                tag="cct")
            nc.scalar.dma_start(out=cct, in_=d_cbc[:, cc, :, :, :])
            bias_cc = cbf_pool.tile([128, 512], F32, name="bias_cc",
                                    tag="bias_cc")
            nc.sync.dma_start(
                out=bias_cc,
                in_=bass.AP(tensor=d_csq, offset=cc * 512,
                            ap=[[0, 128], [1, 512]]))

            if upto == "dma":
                # force the DMAs (tiny reads) without any matmul work
                tch = strips.tile([128, 8], F32, tag="tch")
                nc.vector.tensor_copy(tch, cbt[:, 0, 0:8])
                tc8 = strips.tile([128, 8], F32, tag="tc8")
                nc.vector.tensor_copy(tc8, cct[:, 0, 0, 0:8])
                continue
            # h-outer / token-inner: rhs (the codebook slice) stays FIXED
            # across 8 consecutive MMs; only the strided stationary weights
            # rotate, which HW-measures ~100 ns/MM cheaper than cycling the
            # moving operand every MM. Uses all 8 PSUM banks per half-pass.
            for th in range(TT // 8):
                accs = [psB.tile([128, 512], F32, name=f"acc{j}",
                                 tag=f"acc{j}", bufs=1) for j in range(8)]
                if main_pass:
                    for h in range(HT):
                        for j in range(8):
                            t = th * 8 + j
                            tk, off = t // 4, (t % 4) * 128
                            nc.tensor.matmul(
                                accs[j], lhsT=thr[h][tk][:, off:off + 128],
                                rhs=cbt[:, h, :], start=(h == 0),
                                stop=(not corr_pass and h == HT - 1))
                if corr_pass:
                    for h in range(HT):
                        for j in range(8):
                            t = th * 8 + j
                            tk, off = t // 4, (t % 4) * 128
                            nc.tensor.matmul(
                                accs[j], lhsT=t8[h][tk][:, :, off:off + 128],
                                rhs=cct[:, h, :, :],
                                start=(not main_pass and h == 0),
                                stop=(h == HT - 1), perf_mode=DRM)
                if drain == "none":
                    continue
                for j in range(8):
                    t = th * 8 + j
                    s = strips.tile([128, 512], F32, tag="s")
                    nc.vector.tensor_sub(out=s, in0=accs[j], in1=bias_cc)
                    if argmax:
                        mx8 = strips.tile([128, 8], F32, tag="mx8", bufs=6)
                        nc.vector.max(out=mx8, in_=s)
                        ix8 = strips.tile([128, 8], U32, tag="ix8", bufs=6)
                        nc.vector.max_index(out=ix8, in_max=mx8,
                                            in_values=s)
                        nc.gpsimd.tensor_copy(out=cval[t][:, cc:cc + 1],
                                              in_=mx8[:, 0:1])
                        nc.gpsimd.tensor_copy(out=cidx[t][:, cc:cc + 1],
                                              in_=ix8[:, 0:1])

    # ---------- phase C: combine the 16 chunk winners ----------
    if not argmax:
        return
    with tc.tile_pool(name=f"fin{rep}", bufs=2) as fin:
        for t in range(TT):
            cidxf = fin.tile([128, CCH], F32, tag="cidxf")
            nc.vector.tensor_copy(cidxf, cidx[t])
            gmx = fin.tile([128, 8], F32, tag="gmx")
            nc.vector.max(out=gmx, in_=cval[t])
            mask = fin.tile([128, CCH], F32, tag="mask")
            nc.vector.tensor_scalar(
                out=mask, in0=cval[t], scalar1=gmx[:, 0:1], scalar2=None,
                op0=mybir.AluOpType.is_ge)
            inv = fin.tile([128, CCH], F32, tag="inv")
            nc.vector.tensor_scalar(
                out=inv, in0=mask, scalar1=-16384.0, scalar2=16384.0,
                op0=mybir.AluOpType.mult, op1=mybir.AluOpType.add)
            cand = fin.tile([128, CCH], F32, tag="cand")
            nc.vector.tensor_add(cand, cidxf, chunk_off)
            nc.vector.tensor_add(cand, cand, inv)
            win = fin.tile([128, 1], F32, tag="win")
            nc.vector.tensor_reduce(out=win, in_=cand,
                                    axis=mybir.AxisListType.X,
                                    op=mybir.AluOpType.min)
            lab = fin.tile([128, 1], I32, tag="lab")
            nc.vector.tensor_copy(lab, win)
            nc.sync.dma_start(out=d_lab[:, t:t + 1], in_=lab)


def _round_fp32r(a):
    """RNE to fp32 with 12 low mantissa bits cleared (TRN2 fp32r storage)."""
    u = np.ascontiguousarray(a, np.float32).view(np.uint32)
    half = np.uint32(1 << 11)
    lsb = (u >> 12) & np.uint32(1)
    r = (u + half - np.uint32(1) + lsb) & np.uint32(0xFFFFF000)
    return r.view(np.float32)


def make_in_maps(input_values, ln_weight, ln_bias, proj_weight, codebook):
    input_values = np.ascontiguousarray(input_values, np.float32)
    # 2^13 pre-scale makes the projection produce t * 2^13, matching the
    # native scale of the fp8 DR correction products (see phase B).
    pwt = np.ascontiguousarray(
        np.asarray(proj_weight, np.float32).T * np.float32(SC))
    lnw = np.ascontiguousarray(ln_weight, np.float32)
    lnb = np.ascontiguousarray(ln_bias, np.float32)
    cb = np.ascontiguousarray(codebook, np.float32)

    cbr = _round_fp32r(cb)
    cl = (cb - cbr).astype(np.float32)
    cbc = np.empty((H, 2, C), dtype=ml_dtypes.float8_e4m3)
    cbc[:, 0, :] = (cl * np.float32(SC)).astype(ml_dtypes.float8_e4m3)
    cbc[:, 1, :] = cbr.astype(ml_dtypes.float8_e4m3)
    csq = (0.5 * SC * (cb.astype(np.float64) ** 2).sum(0)).astype(np.float32)
    # chunk-major h-tiled layout: [p, cc, ht, (2,) 512] so each chunk's
    # read is one long contiguous run per partition
    cbr = np.ascontiguousarray(
        cbr.reshape(HT, 128, CCH, 512).transpose(1, 2, 0, 3))
    cbc = np.ascontiguousarray(
        cbc.reshape(HT, 128, 2, CCH, 512).transpose(1, 3, 0, 2, 4))

    in_maps = []
    for i in range(N_CORES):
        in_maps.append({
            "x": np.ascontiguousarray(input_values[i]),
            "pwt": pwt, "lnw": lnw, "lnb": lnb,
            "cbr": cbr, "cbc": cbc, "csq": csq,
        })
    return in_maps


_NC_CACHE = None


def kernel(input_values, ln_weight, ln_bias, proj_weight, codebook):
    global _NC_CACHE
    if _NC_CACHE is None:
        _NC_CACHE = build_nc()
    nc = _NC_CACHE

    in_maps = make_in_maps(input_values, ln_weight, ln_bias, proj_weight,
                           codebook)
    res = run_bass_kernel_spmd(nc, in_maps, core_ids=list(range(N_CORES)))
    out = np.empty((B, L), np.int32)
    for i in range(N_CORES):
        out[i] = res.results[i]["labels"].T.reshape(L)
    return out



# revision 49
# speedup vs baseline: 1.0498x; 1.0498x over previous
"""Trainium2 Bass kernel for BestRQ vector-quantization codebook lookup.

Pipeline (per NeuronCore, data-parallel over batch B=8 across 8 cores):
  x (2048,512) --LayerNorm--> xn --PE transpose--> xnT (d-major)
  t*2^13 = (projW*2^13)^T @ xn^T  (fp32 matmul; 2^13 host-prescale)
  t split: th_r = fp32r(t*2^13) (12-bit mantissa), tl13 = t*2^13 - th_r
  codebook (host-prepped): cbr = fp32r(cb); fp8 pair tensor
    cbc[..,0,:] = e4m3((cb - cbr) * 2^13), cbc[..,1,:] = e4m3(cbr)
  score*2^13 = th_r @ cbr  (fp32r)  +  DR-correction  (fp8 DoubleRow),
  BOTH accumulated into ONE psum bank: the 2^13 prescale of the main
  product matches the native (t.cl + tl.ch)*2^13 scale of the DR pair, so
  no separate correction psum / scalar rescale / DVE add is needed.
  argmax is scale-invariant, so the bias csq13 = 0.5*||c||^2*2^13 is
  subtracted by a single DVE tensor_sub reading PSUM directly, then
  per-chunk argmax via DVE max/max_index; global combine over 16 chunks.

Phase B orders matmuls h-outer/token-inner (8 token tiles x one fixed
codebook rhs slice) across all 8 PSUM banks. The codebook is stored
chunk-major ([128, chunk, htile, cols]) so each 512-column chunk is one
fully-contiguous DMA; the fp32r and fp8 streams ride different hwdge
queues (SP / Activation). NOTE: splitting ONE tile's DMA across the two
queues races (partial-region cross-queue writes) — whole tiles only.

Measured (8-core SPMD, in-NEFF repeat marginal, difference-of-medians
estimator: reproducible to <1 us): body ~1.274 ms/core, at the PE
streaming roofline for the sustained ~2.0 GHz (P0) clock: 4096 N=512 matmuls x ~256 ns + fp32
projection. DMA (~221 us) and all DVE/scalar work are hidden behind the
PE. Phase A pipelines its 64 transposes through 4 PSUM banks (bufs=1
measurably serialized) and runs the t splits on DVE; 3 DVE PSUM-reads
per strip instead of one scalar drain measured ~25 us WORSE.

Numerics: 2 label flips out of 16384 (rel err 1.11e-2), inside the 2e-2
gate. The projection must stay fp32: fp32r+residual-correction variants
were tried (input residuals via fp32r Pe.X + fp8 P.Xe at 2^15, host-
emulated to 4e-6 rms) but HW lands at 4 flips / rel err 1.978e-2 — the
PE's fp32r product-path rounding adds noise input-residual corrections
cannot cancel, leaving <2% gate margin. Not worth 33 us of the 1.29 ms.
"""

import numpy as np
import ml_dtypes

import concourse.bacc as bacc
import concourse.bass as bass
import concourse.mybir as mybir
import concourse.tile as tile
from concourse.bass_utils import run_bass_kernel_spmd
from concourse.masks import make_identity

B, L, D, H, C = 8, 2048, 512, 1024, 8192
LN_EPS = 1e-5
N_CORES = 8

TT = L // 128      # 16 token tiles
CCH = C // 512     # 16 codebook chunks
HT = H // 128      # 8 h tiles
DT = D // 128      # 4 d tiles
TOKC = L // 512    # 4 token chunks (projection)
SC = 8192.0        # 2^13 residual scale

F32 = mybir.dt.float32
F32R = mybir.dt.float32r
F8E4 = mybir.dt.float8e4
I32 = mybir.dt.int32
U32 = mybir.dt.uint32
DRM = mybir.MatmulPerfMode.DoubleRow


def build_nc(corr_pass=True, repeat=1, argmax=True, psb_bufs=5, strip_bufs=8,
             drain="scalar", upto="full", main_pass=True):
    nc = bacc.Bacc("TRN2", target_bir_lowering=False, debug=False)

    d_x = nc.dram_tensor("x", (L, D), F32, kind="ExternalInput")
    d_pwt = nc.dram_tensor("pwt", (D, H), F32, kind="ExternalInput")
    d_lnw = nc.dram_tensor("lnw", (D,), F32, kind="ExternalInput")
    d_lnb = nc.dram_tensor("lnb", (D,), F32, kind="ExternalInput")
    # codebook splits, chunk-major + h-tiled so each chunk is ONE fully
    # contiguous 16KB(8KB)-per-partition DMA read:
    # cbr[p, cc, ht, c] = fp32r(cb)[ht*128+p, cc*512+c]
    d_cbr = nc.dram_tensor("cbr", (128, CCH, HT, 512), F32R,
                           kind="ExternalInput")
    d_cbc = nc.dram_tensor("cbc", (128, CCH, HT, 2, 512), F8E4,
                           kind="ExternalInput")
    d_csq = nc.dram_tensor("csq", (C,), F32, kind="ExternalInput")
    d_lab = nc.dram_tensor("labels", (128, TT), I32, kind="ExternalOutput")

    with tile.TileContext(nc) as tc:
        with tc.tile_pool(name="consts", bufs=1) as consts, \
             tc.tile_pool(name="persist", bufs=1) as persist:

            # ---------- constants ----------
            ident = consts.tile([128, 128], F32)
            make_identity(nc, ident)
            eps_t = consts.tile([128, 1], F32)
            nc.vector.memset(eps_t, LN_EPS)
            lnw_bc = consts.tile([128, D], F32)
            nc.sync.dma_start(
                out=lnw_bc,
                in_=bass.AP(tensor=d_lnw, offset=0, ap=[[0, 128], [1, D]]))
            lnb_bc = consts.tile([128, D], F32)
            nc.sync.dma_start(
                out=lnb_bc,
                in_=bass.AP(tensor=d_lnb, offset=0, ap=[[0, 128], [1, D]]))
            chunk_off = consts.tile([128, CCH], F32)
            for j in range(CCH):
                nc.vector.memset(chunk_off[:, j:j + 1], 512.0 * j)

            # persistent t splits, tiled per (h, tok-chunk) for fine deps
            thr = [[persist.tile([128, 512], F32R, name=f"thr{h}_{tk}",
                                 tag=f"thr{h}_{tk}") for tk in range(TOKC)]
                   for h in range(HT)]
            t8 = [[persist.tile([128, 2, 512], F8E4, name=f"t8_{h}_{tk}",
                                tag=f"t8_{h}_{tk}") for tk in range(TOKC)]
                  for h in range(HT)]
            cval = [persist.tile([128, CCH], F32, name=f"cval{t}",
                                 tag=f"cval{t}") for t in range(TT)]
            cidx = [persist.tile([128, CCH], U32, name=f"cidx{t}",
                                 tag=f"cidx{t}") for t in range(TT)]

            for rep in range(repeat):
                _body(nc, tc, rep, corr_pass, ident, eps_t, lnw_bc, lnb_bc,
                      chunk_off, thr, t8, cval, cidx,
                      d_x, d_pwt, d_cbr, d_cbc, d_csq, d_lab,
                      argmax, psb_bufs, strip_bufs, drain, upto, main_pass)

    nc.compile()
    return nc


def _body(nc, tc, rep, corr_pass, ident, eps_t, lnw_bc, lnb_bc, chunk_off,
          thr, t8, cval, cidx, d_x, d_pwt, d_cbr, d_cbc, d_csq, d_lab,
          argmax=True, psb_bufs=3, strip_bufs=4, drain="scalar", upto="full",
          main_pass=True):
    # ---------- phase A: LN + transposes + projection + split ----------
    with tc.tile_pool(name=f"phA{rep}", bufs=1) as phA, \
         tc.tile_pool(name=f"ldtmp{rep}", bufs=3) as ldtmp, \
         tc.tile_pool(name=f"psA{rep}", bufs=3, space="PSUM") as psA, \
         tc.tile_pool(name=f"psTr{rep}", bufs=4, space="PSUM") as psTr:

        # proj weight, host-pre-transposed to (d, h)
        pwT = [phA.tile([128, H], F32, name=f"pwT{d}", tag=f"pwT{d}")
               for d in range(DT)]
        for d in range(DT):
            nc.sync.dma_start(out=pwT[d], in_=d_pwt[d * 128:(d + 1) * 128, :])

        # LayerNorm + transpose to xnT (d, tok)
        xnT = [phA.tile([128, L], F32, name=f"xnT{d}", tag=f"xnT{d}")
               for d in range(DT)]
        for t in range(TT):
            x_t = ldtmp.tile([128, D], F32, tag="x_t")
            nc.sync.dma_start(out=x_t, in_=d_x[t * 128:(t + 1) * 128, :])
            stats = ldtmp.tile([128, 6], F32, tag="stats")
            nc.vector.bn_stats(out=stats, in_=x_t)
            mv = ldtmp.tile([128, 2], F32, tag="mv")
            nc.vector.bn_aggr(out=mv, in_=stats)
            rstd = ldtmp.tile([128, 1], F32, tag="rstd")
            nc.scalar.activation(out=rstd, in_=mv[:, 1:2],
                                 func=mybir.ActivationFunctionType.Sqrt,
                                 bias=eps_t, scale=1.0)
            nc.vector.reciprocal(out=rstd, in_=rstd)
            xn = ldtmp.tile([128, D], F32, tag="xn")
            nc.vector.tensor_scalar(
                out=xn, in0=x_t, scalar1=mv[:, 0:1], scalar2=rstd,
                op0=mybir.AluOpType.subtract, op1=mybir.AluOpType.mult)
            nc.vector.tensor_mul(out=xn, in0=xn, in1=lnw_bc)
            nc.vector.tensor_add(out=xn, in0=xn, in1=lnb_bc)
            for d in range(DT):
                ps_tr = psTr.tile([128, 128], F32, tag="ps_tr")
                nc.tensor.transpose(ps_tr, xn[:, d * 128:(d + 1) * 128],
                                    ident)
                nc.scalar.copy(out=xnT[d][:, t * 128:(t + 1) * 128],
                               in_=ps_tr)

        # projection t^T[h_tile, tok] = sum_d pwT[d,h].T @ xnT[d, tok]
        # fp32 (exact); split each PSUM strip into fp32r + scaled-fp8.
        for tk in range(TOKC):
            for h in range(HT):
                ps_t = psA.tile([128, 512], F32, tag="ps_t")
                for d in range(DT):
                    nc.tensor.matmul(
                        ps_t,
                        lhsT=pwT[d][:, h * 128:(h + 1) * 128],
                        rhs=xnT[d][:, tk * 512:(tk + 1) * 512],
                        start=(d == 0), stop=(d == DT - 1))
                # pwt is host-scaled by 2^13, so t_f = t * 2^13. A single
                # scalar read drains the PSUM bank; the split runs on DVE
                # from the SBUF copy (3 DVE PSUM-reads instead measurably
                # hold the bank too long and cost ~25 us).
                t_f = ldtmp.tile([128, 512], F32, tag="t_f")
                nc.scalar.copy(out=t_f, in_=ps_t)
                # th_r = fp32r(t * 2^13); the split only needs
                # self-consistency (t8[0] is computed against stored thr)
                nc.vector.tensor_copy(thr[h][tk], t_f)
                # t8[:,1,:] = e4m3(t) (pair slot 1 multiplies ch8)
                nc.vector.tensor_scalar(
                    out=t8[h][tk][:, 1, :], in0=t_f, scalar1=1.0 / SC,
                    scalar2=None, op0=mybir.AluOpType.mult)
                # tl * 2^13 = t_f - th_r, exact in fp32
                tlt = ldtmp.tile([128, 512], F32, tag="tlt")
                nc.vector.tensor_sub(out=tlt, in0=t_f,
                                     in1=thr[h][tk].bitcast(F32))
                # t8[:,0,:] = e4m3(tl * 2^13) (pair slot 0, times cl8s)
                nc.scalar.copy(out=t8[h][tk][:, 0, :], in_=tlt)

    if upto == "A":
        return
    # ---------- phase B: cross matmul + per-chunk argmax ----------
    # Main fp32r matmuls produce t.c * 2^13 (pwt host-scaled); the fp8 DR
    # correction matmuls natively produce (t.cl + tl.ch) * 2^13, so both
    # accumulate into ONE psum bank. argmax is scale-invariant, so the bias
    # is host-prescaled (csq13 = 0.5*||c||^2 * 2^13) and never rescaled:
    # the only drain is a single DVE sub reading PSUM.
    with tc.tile_pool(name=f"cbf{rep}", bufs=2) as cbf_pool, \
         tc.tile_pool(name=f"strips{rep}", bufs=strip_bufs) as strips, \
         tc.tile_pool(name=f"psB{rep}", bufs=psb_bufs, space="PSUM") as psB:

        for cc in range(CCH):
            # split each chunk's streams across BOTH hwdge queues (SP and
            # Activation) — a single queue's bandwidth gates the kernel
            cbt = cbf_pool.tile([128, HT, 512], F32R, name="cbt", tag="cbt")
            nc.sync.dma_start(out=cbt, in_=d_cbr[:, cc, :, :])
            cct = cbf_pool.tile([128, HT, 2, 512], F8E4, name="cct",
                                tag="cct")
            nc.scalar.dma_start(out=cct, in_=d_cbc[:, cc, :, :, :])
            bias_cc = cbf_pool.tile([128, 512], F32, name="bias_cc",
                                    tag="bias_cc")
            nc.sync.dma_start(
                out=bias_cc,
                in_=bass.AP(tensor=d_csq, offset=cc * 512,
                            ap=[[0, 128], [1, 512]]))

            if upto == "dma":
                # force the DMAs (tiny reads) without any matmul work
                tch = strips.tile([128, 8], F32, tag="tch")
                nc.vector.tensor_copy(tch, cbt[:, 0, 0:8])
                tc8 = strips.tile([128, 8], F32, tag="tc8")
                nc.vector.tensor_copy(tc8, cct[:, 0, 0, 0:8])
                continue
            # h-outer / token-inner: rhs (the codebook slice) stays FIXED
            # across 8 consecutive MMs; only the strided stationary weights
            # rotate, which HW-measures ~100 ns/MM cheaper than cycling the
            # moving operand every MM. Uses all 8 PSUM banks per half-pass.
            for th in range(TT // 8):
                accs = [psB.tile([128, 512], F32, name=f"acc{j}",
                                 tag=f"acc{j}", bufs=1) for j in range(8)]
                if main_pass:
                    for h in range(HT):
                        for j in range(8):
                            t = th * 8 + j
                            tk, off = t // 4, (t % 4) * 128
                            nc.tensor.matmul(
                                accs[j], lhsT=thr[h][tk][:, off:off + 128],
                                rhs=cbt[:, h, :], start=(h == 0),
                                stop=(not corr_pass and h == HT - 1))
                if corr_pass:
                    for h in range(HT):
                        for j in range(8):
                            t = th * 8 + j
                            tk, off = t // 4, (t % 4) * 128
                            nc.tensor.matmul(
                                accs[j], lhsT=t8[h][tk][:, :, off:off + 128],
                                rhs=cct[:, h, :, :],
                                start=(not main_pass and h == 0),
                                stop=(h == HT - 1), perf_mode=DRM)
                if drain == "none":
                    continue
                for j in range(8):
                    t = th * 8 + j
                    s = strips.tile([128, 512], F32, tag="s")
                    nc.vector.tensor_sub(out=s, in0=accs[j], in1=bias_cc)
                    if argmax:
                        mx8 = strips.tile([128, 8], F32, tag="mx8", bufs=6)
                        nc.vector.max(out=mx8, in_=s)
                        ix8 = strips.tile([128, 8], U32, tag="ix8", bufs=6)
                        nc.vector.max_index(out=ix8, in_max=mx8,
                                            in_values=s)
                        nc.gpsimd.tensor_copy(out=cval[t][:, cc:cc + 1],
                                              in_=mx8[:, 0:1])
                        nc.gpsimd.tensor_copy(out=cidx[t][:, cc:cc + 1],
                                              in_=ix8[:, 0:1])

    # ---------- phase C: combine the 16 chunk winners ----------
    if not argmax:
        return
    with tc.tile_pool(name=f"fin{rep}", bufs=2) as fin:
        for t in range(TT):
            cidxf = fin.tile([128, CCH], F32, tag="cidxf")
            nc.vector.tensor_copy(cidxf, cidx[t])
            gmx = fin.tile([128, 8], F32, tag="gmx")
            nc.vector.max(out=gmx, in_=cval[t])
            mask = fin.tile([128, CCH], F32, tag="mask")
            nc.vector.tensor_scalar(
                out=mask, in0=cval[t], scalar1=gmx[:, 0:1], scalar2=None,
                op0=mybir.AluOpType.is_ge)
            inv = fin.tile([128, CCH], F32, tag="inv")
            nc.vector.tensor_scalar(
                out=inv, in0=mask, scalar1=-16384.0, scalar2=16384.0,
                op0=mybir.AluOpType.mult, op1=mybir.AluOpType.add)
            cand = fin.tile([128, CCH], F32, tag="cand")
            nc.vector.tensor_add(cand, cidxf, chunk_off)
            nc.vector.tensor_add(cand, cand, inv)
            win = fin.tile([128, 1], F32, tag="win")
            nc.vector.tensor_reduce(out=win, in_=cand,
                                    axis=mybir.AxisListType.X,
                                    op=mybir.AluOpType.min)
            lab = fin.tile([128, 1], I32, tag="lab")
            nc.vector.tensor_copy(lab, win)
            nc.sync.dma_start(out=d_lab[:, t:t + 1], in_=lab)


def _round_fp32r(a):
    """RNE to fp32 with 12 low mantissa bits cleared (TRN2 fp32r storage)."""
    u = np.ascontiguousarray(a, np.float32).view(np.uint32)
    half = np.uint32(1 << 11)
    lsb = (u >> 12) & np.uint32(1)
    r = (u + half - np.uint32(1) + lsb) & np.uint32(0xFFFFF000)
    return r.view(np.float32)


def make_in_maps(input_values, ln_weight, ln_bias, proj_weight, codebook):
    input_values = np.ascontiguousarray(input_values, np.float32)
    # 2^13 pre-scale makes the projection produce t * 2^13, matching the
    # native scale of the fp8 DR correction products (see phase B).
    pwt = np.ascontiguousarray(
        np.asarray(proj_weight, np.float32).T * np.float32(SC))
    lnw = np.ascontiguousarray(ln_weight, np.float32)
    lnb = np.ascontiguousarray(ln_bias, np.float32)
    cb = np.ascontiguousarray(codebook, np.float32)

    cbr = _round_fp32r(cb)
    cl = (cb - cbr).astype(np.float32)
    cbc = np.empty((H, 2, C), dtype=ml_dtypes.float8_e4m3)
    cbc[:, 0, :] = (cl * np.float32(SC)).astype(ml_dtypes.float8_e4m3)
    cbc[:, 1, :] = cbr.astype(ml_dtypes.float8_e4m3)
    csq = (0.5 * SC * (cb.astype(np.float64) ** 2).sum(0)).astype(np.float32)
    # chunk-major h-tiled layout: [p, cc, ht, (2,) 512] so each chunk's
    # read is one long contiguous run per partition
    cbr = np.ascontiguousarray(
        cbr.reshape(HT, 128, CCH, 512).transpose(1, 2, 0, 3))
    cbc = np.ascontiguousarray(
        cbc.reshape(HT, 128, 2, CCH, 512).transpose(1, 3, 0, 2, 4))

    in_maps = []
    for i in range(N_CORES):
        in_maps.append({
            "x": np.ascontiguousarray(input_values[i]),
            "pwt": pwt, "lnw": lnw, "lnb": lnb,
            "cbr": cbr, "cbc": cbc, "csq": csq,
        })
    return in_maps


_NC_CACHE = None


def kernel(input_values, ln_weight, ln_bias, proj_weight, codebook):
    global _NC_CACHE
    if _NC_CACHE is None:
        _NC_CACHE = build_nc()
    nc = _NC_CACHE

    in_maps = make_in_maps(input_values, ln_weight, ln_bias, proj_weight,
                           codebook)
    res = run_bass_kernel_spmd(nc, in_maps, core_ids=list(range(N_CORES)))
    out = np.empty((B, L), np.int32)
    for i in range(N_CORES):
        out[i] = res.results[i]["labels"].T.reshape(L)
    return out

